# revision 4
# baseline (speedup 1.0000x reference)
"""Trainium2 Bass kernel for nn_Cache_28071906246843 (retrieval_knn).

reference semantics:
    q = h_t[cache_words]                         # [C, D] gather
    dist = sqrt(sum((cache_h - q)**2, -1))       # [C]
    vals = exp(dist / 32.0)                      # [C]
    cache_p = segment_sum(vals, cache_words, V)  # [V]
    out = log_softmax(cache_p[None, :])          # [1, V]

Sharding: cache elements are sorted by word id (pure reordering, improves
gather locality) and split into 8 contiguous shards of 16384 elements, one
per NeuronCore.  Each core streams its cache_h shard tile-by-tile
([128, 1024] tiles), gathers the matching h_t rows with an indirect DMA,
and computes the per-element kernel value exp(||h-q||/32) on device
(DVE subtract, ACT square+accumulate, sqrt, exp).  The [V] segment-sum of
8*16384 scalars and the final log_softmax over [V] are done on the host.
"""

import sys

import numpy as np

if "/opt/trn_rl_repo" not in sys.path:
    sys.path.insert(0, "/opt/trn_rl_repo")

import concourse.bass as bass
import concourse.tile as tile
from concourse import bacc, mybir
from concourse.bass_utils import run_bass_kernel_spmd

V, D, C = 50257, 1024, 131072
NCORES = 8
CSH = C // NCORES  # 16384 elements per core
P = 128            # SBUF partitions
NT = CSH // P      # 128 tiles per core
SMOOTH = 32.0


def build_nc(nt: int = NT, v: int = V, d: int = D) -> bass.Bass:
    """Build the per-core Bass program (SPMD: same program on all cores)."""
    nc = bacc.Bacc(
        "TRN2", target_bir_lowering=False, debug=False, num_devices=NCORES
    )
    ht = nc.dram_tensor("ht", [v, d], mybir.dt.float32, kind="ExternalInput")
    ch = nc.dram_tensor("ch", [nt * P, d], mybir.dt.float32, kind="ExternalInput")
    # cw is pre-transposed on host: cw[p, t] = word id of element t*128+p
    cw = nc.dram_tensor("cw", [P, nt], mybir.dt.int32, kind="ExternalInput")
    vals = nc.dram_tensor("vals", [P, nt], mybir.dt.float32, kind="ExternalOutput")

    ch_t = ch.ap().rearrange("(t p) d -> t p d", p=P)  # [nt, 128, d]

    with tile.TileContext(nc) as tc:
        with (
            tc.tile_pool(name="io", bufs=4) as io,
            tc.tile_pool(name="scratch", bufs=2) as scratch,
            tc.tile_pool(name="stats", bufs=8) as stats,
            tc.tile_pool(name="persist", bufs=1) as persist,
        ):
            cw_sb = persist.tile([P, nt], mybir.dt.int32)
            nc.sync.dma_start(out=cw_sb[:], in_=cw.ap())
            vals_sb = persist.tile([P, nt], mybir.dt.float32)

            for t in range(nt):
                ch_tile = io.tile([P, d], mybir.dt.float32, tag="ch")
                nc.sync.dma_start(out=ch_tile[:], in_=ch_t[t])

                q_tile = io.tile([P, d], mybir.dt.float32, tag="q")
                nc.gpsimd.indirect_dma_start(
                    out=q_tile[:],
                    out_offset=None,
                    in_=ht.ap(),
                    in_offset=bass.IndirectOffsetOnAxis(
                        ap=cw_sb[:, t : t + 1], axis=0
                    ),
                )

                d_tile = io.tile([P, d], mybir.dt.float32, tag="d")
                nc.vector.tensor_tensor(
                    out=d_tile[:],
                    in0=ch_tile[:],
                    in1=q_tile[:],
                    op=mybir.AluOpType.subtract,
                )

                sq_tile = scratch.tile([P, d], mybir.dt.float32, tag="sq")
                d2 = stats.tile([P, 1], mybir.dt.float32, tag="d2")
                nc.scalar.activation(
                    out=sq_tile[:],
                    in_=d_tile[:],
                    func=mybir.ActivationFunctionType.Square,
                    accum_out=d2[:],
                )
                dist = stats.tile([P, 1], mybir.dt.float32, tag="dist")
                nc.scalar.activation(
                    out=dist[:],
                    in_=d2[:],
                    func=mybir.ActivationFunctionType.Sqrt,
                )
                nc.scalar.activation(
                    out=vals_sb[:, t : t + 1],
                    in_=dist[:],
                    func=mybir.ActivationFunctionType.Exp,
                    scale=1.0 / SMOOTH,
                )

            nc.sync.dma_start(out=vals.ap(), in_=vals_sb[:])
    nc.compile()
    return nc


def make_in_maps(h_t, ch_sorted, cw_sorted):
    in_maps = []
    for c in range(NCORES):
        sl = slice(c * CSH, (c + 1) * CSH)
        in_maps.append(
            {
                "ht": h_t,
                "ch": ch_sorted[sl],
                "cw": np.ascontiguousarray(cw_sorted[sl].reshape(NT, P).T),
            }
        )
    return in_maps


def finish_on_host(vals_sorted, cw_sorted):
    """segment-sum + log_softmax (tiny O(C)+O(V) work)."""
    p = np.bincount(cw_sorted, weights=vals_sorted.astype(np.float64), minlength=V)
    m = p.max()
    lse = m + np.log(np.exp(p - m).sum())
    return (p - lse).astype(np.float32)[None, :]


def _prep(h_t, cache_h, cache_words):
    h_t = np.ascontiguousarray(np.asarray(h_t), dtype=np.float32)
    cache_h = np.ascontiguousarray(np.asarray(cache_h), dtype=np.float32)
    cw = np.asarray(cache_words).astype(np.int32)
    order = np.argsort(cw, kind="stable")
    return h_t, cache_h[order], cw[order]


def kernel(h_t, cache_h, cache_words):
    h_t, ch_sorted, cw_sorted = _prep(h_t, cache_h, cache_words)
    nc = build_nc()
    res = run_bass_kernel_spmd(
        nc, make_in_maps(h_t, ch_sorted, cw_sorted), core_ids=list(range(NCORES))
    )
    vals_sorted = np.concatenate(
        [r["vals"].T.reshape(-1) for r in res.results]
    )
    return finish_on_host(vals_sorted, cw_sorted)


# revision 7
# speedup vs baseline: 1.3849x; 1.3849x over previous
"""Trainium2 Bass kernel for nn_Cache_28071906246843 (retrieval_knn).

reference semantics:
    q = h_t[cache_words]                         # [C, D] gather
    dist = sqrt(sum((cache_h - q)**2, -1))       # [C]
    vals = exp(dist / 32.0)                      # [C]
    cache_p = segment_sum(vals, cache_words, V)  # [V]
    out = log_softmax(cache_p[None, :])          # [1, V]

Sharding: cache elements are sorted by word id (pure reordering, improves
gather locality) and split into 8 contiguous shards of 16384 elements, one
per NeuronCore.  Each core streams its cache_h shard tile-by-tile
([128, 1024] tiles), gathers the matching h_t rows with an indirect DMA,
and computes the per-element kernel value exp(||h-q||/32) on device
(DVE subtract, ACT square+accumulate, sqrt, exp).  The [V] segment-sum of
8*16384 scalars and the final log_softmax over [V] are done on the host.
"""

import sys

import numpy as np

if "/opt/trn_rl_repo" not in sys.path:
    sys.path.insert(0, "/opt/trn_rl_repo")

import concourse.bass as bass
import concourse.tile as tile
from concourse import bacc, mybir
from concourse.bass_utils import run_bass_kernel_spmd

V, D, C = 50257, 1024, 131072
NCORES = 8
CSH = C // NCORES  # 16384 elements per core
P = 128            # SBUF partitions
NT = CSH // P      # 128 tiles per core
SMOOTH = 32.0


def build_nc(nt: int = NT, v: int = V, d: int = D) -> bass.Bass:
    """Build the per-core Bass program (SPMD: same program on all cores)."""
    nc = bacc.Bacc(
        "TRN2", target_bir_lowering=False, debug=False, num_devices=NCORES
    )
    ht = nc.dram_tensor("ht", [v, d], mybir.dt.float32, kind="ExternalInput")
    ch = nc.dram_tensor("ch", [nt * P, d], mybir.dt.float32, kind="ExternalInput")
    # cw is pre-transposed on host: cw[p, t] = word id of element t*128+p
    cw = nc.dram_tensor("cw", [P, nt], mybir.dt.int32, kind="ExternalInput")
    vals = nc.dram_tensor("vals", [P, nt], mybir.dt.float32, kind="ExternalOutput")

    ch_t = ch.ap().rearrange("(t p) d -> t p d", p=P)  # [nt, 128, d]

    with tile.TileContext(nc) as tc:
        with (
            tc.tile_pool(name="io", bufs=6) as io,
            tc.tile_pool(name="scratch", bufs=2) as scratch,
            tc.tile_pool(name="persist", bufs=1) as persist,
        ):
            cw_sb = persist.tile([P, nt], mybir.dt.int32)
            nc.sync.dma_start(out=cw_sb[:], in_=cw.ap())
            vals_sb = persist.tile([P, nt], mybir.dt.float32)
            d2_all = persist.tile([P, nt], mybir.dt.float32)

            for t in range(nt):
                ch_tile = io.tile([P, d], mybir.dt.float32, tag="ch")
                nc.sync.dma_start(out=ch_tile[:], in_=ch_t[t])

                q_tile = io.tile([P, d], mybir.dt.float32, tag="q")
                nc.gpsimd.indirect_dma_start(
                    out=q_tile[:],
                    out_offset=None,
                    in_=ht.ap(),
                    in_offset=bass.IndirectOffsetOnAxis(
                        ap=cw_sb[:, t : t + 1], axis=0
                    ),
                )

                d_tile = io.tile([P, d], mybir.dt.float32, tag="d")
                nc.vector.tensor_tensor(
                    out=d_tile[:],
                    in0=ch_tile[:],
                    in1=q_tile[:],
                    op=mybir.AluOpType.subtract,
                )

                sq_tile = scratch.tile([P, d], mybir.dt.float32, tag="sq")
                nc.scalar.activation(
                    out=sq_tile[:],
                    in_=d_tile[:],
                    func=mybir.ActivationFunctionType.Square,
                    accum_out=d2_all[:, t : t + 1],
                )

            # batched tail: one table switch each instead of two per tile
            dist_all = persist.tile([P, nt], mybir.dt.float32)
            nc.scalar.activation(
                out=dist_all[:],
                in_=d2_all[:],
                func=mybir.ActivationFunctionType.Sqrt,
            )
            nc.scalar.activation(
                out=vals_sb[:],
                in_=dist_all[:],
                func=mybir.ActivationFunctionType.Exp,
                scale=1.0 / SMOOTH,
            )

            nc.sync.dma_start(out=vals.ap(), in_=vals_sb[:])
    nc.compile()
    return nc


def make_in_maps(h_t, ch_sorted, cw_sorted):
    in_maps = []
    for c in range(NCORES):
        sl = slice(c * CSH, (c + 1) * CSH)
        in_maps.append(
            {
                "ht": h_t,
                "ch": ch_sorted[sl],
                "cw": np.ascontiguousarray(cw_sorted[sl].reshape(NT, P).T),
            }
        )
    return in_maps


def finish_on_host(vals_sorted, cw_sorted):
    """segment-sum + log_softmax (tiny O(C)+O(V) work)."""
    p = np.bincount(cw_sorted, weights=vals_sorted.astype(np.float64), minlength=V)
    m = p.max()
    lse = m + np.log(np.exp(p - m).sum())
    return (p - lse).astype(np.float32)[None, :]


def _prep(h_t, cache_h, cache_words):
    h_t = np.ascontiguousarray(np.asarray(h_t), dtype=np.float32)
    cache_h = np.ascontiguousarray(np.asarray(cache_h), dtype=np.float32)
    cw = np.asarray(cache_words).astype(np.int32)
    order = np.argsort(cw, kind="stable")
    return h_t, cache_h[order], cw[order]


def kernel(h_t, cache_h, cache_words):
    h_t, ch_sorted, cw_sorted = _prep(h_t, cache_h, cache_words)
    nc = build_nc()
    res = run_bass_kernel_spmd(
        nc, make_in_maps(h_t, ch_sorted, cw_sorted), core_ids=list(range(NCORES))
    )
    vals_sorted = np.concatenate(
        [r["vals"].T.reshape(-1) for r in res.results]
    )
    return finish_on_host(vals_sorted, cw_sorted)


# revision 9
# speedup vs baseline: 1.5792x; 1.1403x over previous
"""Trainium2 Bass kernel for nn_Cache_28071906246843 (retrieval_knn).

reference semantics:
    q = h_t[cache_words]                         # [C, D] gather
    dist = sqrt(sum((cache_h - q)**2, -1))       # [C]
    vals = exp(dist / 32.0)                      # [C]
    cache_p = segment_sum(vals, cache_words, V)  # [V]
    out = log_softmax(cache_p[None, :])          # [1, V]

Sharding: cache elements are sorted by word id (pure reordering, improves
gather locality) and split into 8 contiguous shards of 16384 elements, one
per NeuronCore.  Each core streams its cache_h shard tile-by-tile
([128, 1024] tiles), gathers the matching h_t rows with an indirect DMA,
and computes the per-element kernel value exp(||h-q||/32) on device
(DVE subtract, ACT square+accumulate, sqrt, exp).  The [V] segment-sum of
8*16384 scalars and the final log_softmax over [V] are done on the host.
"""

import sys

import numpy as np

if "/opt/trn_rl_repo" not in sys.path:
    sys.path.insert(0, "/opt/trn_rl_repo")

import concourse.bass as bass
import concourse.tile as tile
from concourse import bacc, mybir
from concourse.bass_utils import run_bass_kernel_spmd

V, D, C = 50257, 1024, 131072
NCORES = 8
CSH = C // NCORES  # 16384 elements per core
P = 128            # SBUF partitions
NT = CSH // P      # 128 tiles per core
SMOOTH = 32.0


def build_nc(nt: int = NT, v: int = V, d: int = D) -> bass.Bass:
    """Build the per-core Bass program (SPMD: same program on all cores)."""
    nc = bacc.Bacc(
        "TRN2", target_bir_lowering=False, debug=False, num_devices=NCORES
    )
    ht = nc.dram_tensor("ht", [v, d], mybir.dt.float32, kind="ExternalInput")
    ch = nc.dram_tensor("ch", [nt * P, d], mybir.dt.float32, kind="ExternalInput")
    # cw is pre-transposed on host: cw[p, t] = word id of element t*128+p
    cw = nc.dram_tensor("cw", [P, nt], mybir.dt.int32, kind="ExternalInput")
    vals = nc.dram_tensor("vals", [P, nt], mybir.dt.float32, kind="ExternalOutput")

    ch_t = ch.ap().rearrange("(t p) d -> t p d", p=P)  # [nt, 128, d]

    with tile.TileContext(nc) as tc:
        with (
            tc.tile_pool(name="io", bufs=6) as io,
            tc.tile_pool(name="scratch", bufs=2) as scratch,
            tc.tile_pool(name="persist", bufs=1) as persist,
        ):
            cw_sb = persist.tile([P, nt], mybir.dt.int32)
            nc.sync.dma_start(out=cw_sb[:], in_=cw.ap())
            vals_sb = persist.tile([P, nt], mybir.dt.float32)
            d2_all = persist.tile([P, nt], mybir.dt.float32)

            for t in range(nt):
                ch_tile = io.tile([P, d], mybir.dt.float32, tag="ch")
                nc.sync.dma_start(out=ch_tile[:], in_=ch_t[t])

                q_tile = io.tile([P, d], mybir.dt.float32, tag="q")
                nc.gpsimd.indirect_dma_start(
                    out=q_tile[:],
                    out_offset=None,
                    in_=ht.ap(),
                    in_offset=bass.IndirectOffsetOnAxis(
                        ap=cw_sb[:, t : t + 1], axis=0
                    ),
                )

                d_tile = io.tile([P, d], mybir.dt.float32, tag="d")
                nc.vector.tensor_tensor(
                    out=d_tile[:],
                    in0=ch_tile[:],
                    in1=q_tile[:],
                    op=mybir.AluOpType.subtract,
                )

                sq_tile = scratch.tile([P, d], mybir.dt.float32, tag="sq")
                nc.scalar.activation(
                    out=sq_tile[:],
                    in_=d_tile[:],
                    func=mybir.ActivationFunctionType.Square,
                    accum_out=d2_all[:, t : t + 1],
                )

            # batched tail: one table switch each instead of two per tile
            dist_all = persist.tile([P, nt], mybir.dt.float32)
            nc.scalar.activation(
                out=dist_all[:],
                in_=d2_all[:],
                func=mybir.ActivationFunctionType.Sqrt,
            )
            nc.scalar.activation(
                out=vals_sb[:],
                in_=dist_all[:],
                func=mybir.ActivationFunctionType.Exp,
                scale=1.0 / SMOOTH,
            )

            nc.sync.dma_start(out=vals.ap(), in_=vals_sb[:])
    nc.compile()
    return nc


SUP = 2            # element-tiles per supertile
NSUP = NT // SUP   # 64 supertiles per core
SUPW = SUP * P     # 256 elements per supertile


def build_nc_v4(nt: int = NT, v: int = V, d: int = D) -> bass.Bass:
    """Dedup-gather variant: per supertile (256 sorted elements) gather the
    <=128 distinct h_t rows once, then expand to per-element rows with a
    one-hot matmul on the TensorEngine (Q = S^T @ W, S[w,e] = [rel[e]==w])."""
    nsup = nt // SUP
    nc = bacc.Bacc(
        "TRN2", target_bir_lowering=False, debug=False, num_devices=NCORES
    )
    ht = nc.dram_tensor("ht", [v, d], mybir.dt.float32, kind="ExternalInput")
    ch = nc.dram_tensor("ch", [nt * P, d], mybir.dt.float32, kind="ExternalInput")
    # widx[p, s] = p-th (padded) distinct word id of supertile s
    widx = nc.dram_tensor("widx", [P, nsup], mybir.dt.int32, kind="ExternalInput")
    # rel[s, k, e] = index of element (2s+k)*128+e's word within supertile s's
    # distinct-word list
    rel = nc.dram_tensor("rel", [nsup, SUP, P], mybir.dt.int32, kind="ExternalInput")
    vals = nc.dram_tensor("vals", [P, nt], mybir.dt.float32, kind="ExternalOutput")

    ch_t = ch.ap().rearrange("(t p) d -> t p d", p=P)  # [nt, 128, d]

    with tile.TileContext(nc) as tc:
        with (
            tc.tile_pool(name="io", bufs=6) as io,
            tc.tile_pool(name="wpool", bufs=3) as wpool,
            tc.tile_pool(name="spool", bufs=6) as spool,
            tc.tile_pool(name="psum", bufs=2, space="PSUM") as psum,
            tc.tile_pool(name="scratch", bufs=2) as scratch,
            tc.tile_pool(name="persist", bufs=1) as persist,
        ):
            widx_sb = persist.tile([P, nsup], mybir.dt.int32)
            nc.sync.dma_start(out=widx_sb[:], in_=widx.ap())
            iota_sb = persist.tile([P, 1], mybir.dt.int32)
            nc.gpsimd.iota(iota_sb[:], pattern=[[0, 1]], base=0, channel_multiplier=1)
            vals_sb = persist.tile([P, nt], mybir.dt.float32)
            d2_all = persist.tile([P, nt], mybir.dt.float32)

            for s in range(nsup):
                w_f32 = wpool.tile([P, d], mybir.dt.float32, tag="wf")
                nc.gpsimd.indirect_dma_start(
                    out=w_f32[:],
                    out_offset=None,
                    in_=ht.ap(),
                    in_offset=bass.IndirectOffsetOnAxis(
                        ap=widx_sb[:, s : s + 1], axis=0
                    ),
                )
                w_bf = wpool.tile([P, d], mybir.dt.bfloat16, tag="wbf")
                nc.scalar.copy(out=w_bf[:], in_=w_f32[:])

                for k in range(SUP):
                    t = SUP * s + k
                    ch_tile = io.tile([P, d], mybir.dt.float32, tag="ch")
                    nc.sync.dma_start(out=ch_tile[:], in_=ch_t[t])

                    # replicate rel row across all 128 partitions via DMA
                    rel_bc = spool.tile([P, P], mybir.dt.int32, tag="rel")
                    rel_row = bass.AP(
                        tensor=rel.ap().tensor,
                        offset=(s * SUP + k) * P,
                        ap=[[0, P], [1, P]],
                    )
                    nc.gpsimd.dma_start(out=rel_bc[:], in_=rel_row)

                    s_onehot = spool.tile([P, P], mybir.dt.bfloat16, tag="sel")
                    nc.vector.tensor_tensor(
                        out=s_onehot[:],
                        in0=rel_bc[:],
                        in1=iota_sb[:].to_broadcast([P, P]),
                        op=mybir.AluOpType.is_equal,
                    )

                    q_psum = psum.tile([P, d], mybir.dt.float32)
                    for h in range(0, d, 512):
                        nc.tensor.matmul(
                            out=q_psum[:, h : h + 512],
                            lhsT=s_onehot[:],
                            rhs=w_bf[:, h : h + 512],
                            start=True,
                            stop=True,
                        )

                    d_tile = io.tile([P, d], mybir.dt.float32, tag="d")
                    nc.vector.tensor_tensor(
                        out=d_tile[:],
                        in0=ch_tile[:],
                        in1=q_psum[:],
                        op=mybir.AluOpType.subtract,
                    )

                    sq_tile = scratch.tile([P, d], mybir.dt.float32, tag="sq")
                    nc.scalar.activation(
                        out=sq_tile[:],
                        in_=d_tile[:],
                        func=mybir.ActivationFunctionType.Square,
                        accum_out=d2_all[:, t : t + 1],
                    )

            dist_all = persist.tile([P, nt], mybir.dt.float32)
            nc.scalar.activation(
                out=dist_all[:],
                in_=d2_all[:],
                func=mybir.ActivationFunctionType.Sqrt,
            )
            nc.scalar.activation(
                out=vals_sb[:],
                in_=dist_all[:],
                func=mybir.ActivationFunctionType.Exp,
                scale=1.0 / SMOOTH,
            )
            nc.sync.dma_start(out=vals.ap(), in_=vals_sb[:])
    nc.compile()
    return nc


def prep_v4(cw_sorted):
    """Per-core supertile metadata. Returns None if any supertile has more
    than 128 distinct words (fall back to per-element gather then)."""
    widx_all, rel_all = [], []
    for c in range(NCORES):
        shard = cw_sorted[c * CSH : (c + 1) * CSH]
        widx = np.empty((NSUP, P), np.int32)
        rel = np.empty((NSUP, SUP, P), np.int32)
        for s in range(NSUP):
            seg = shard[s * SUPW : (s + 1) * SUPW]
            uw = np.unique(seg)
            if len(uw) > P:
                return None
            widx[s, : len(uw)] = uw
            widx[s, len(uw) :] = uw[-1]
            rel[s] = np.searchsorted(uw, seg).reshape(SUP, P).astype(np.int32)
        widx_all.append(np.ascontiguousarray(widx.T))
        rel_all.append(rel)
    return widx_all, rel_all


def make_in_maps_v4(h_t, ch_sorted, widx_all, rel_all):
    in_maps = []
    for c in range(NCORES):
        sl = slice(c * CSH, (c + 1) * CSH)
        in_maps.append(
            {
                "ht": h_t,
                "ch": ch_sorted[sl],
                "widx": widx_all[c],
                "rel": rel_all[c],
            }
        )
    return in_maps


def make_in_maps(h_t, ch_sorted, cw_sorted):
    in_maps = []
    for c in range(NCORES):
        sl = slice(c * CSH, (c + 1) * CSH)
        in_maps.append(
            {
                "ht": h_t,
                "ch": ch_sorted[sl],
                "cw": np.ascontiguousarray(cw_sorted[sl].reshape(NT, P).T),
            }
        )
    return in_maps


def finish_on_host(vals_sorted, cw_sorted):
    """segment-sum + log_softmax (tiny O(C)+O(V) work)."""
    p = np.bincount(cw_sorted, weights=vals_sorted.astype(np.float64), minlength=V)
    m = p.max()
    lse = m + np.log(np.exp(p - m).sum())
    return (p - lse).astype(np.float32)[None, :]


def _prep(h_t, cache_h, cache_words):
    h_t = np.ascontiguousarray(np.asarray(h_t), dtype=np.float32)
    cache_h = np.ascontiguousarray(np.asarray(cache_h), dtype=np.float32)
    cw = np.asarray(cache_words).astype(np.int32)
    order = np.argsort(cw, kind="stable")
    return h_t, cache_h[order], cw[order]


def run_device(h_t, ch_sorted, cw_sorted, force_v1=False):
    """Compile + run the SPMD program; returns per-element vals (sorted order)."""
    v4 = None if force_v1 else prep_v4(cw_sorted)
    if v4 is not None:
        nc = build_nc_v4()
        in_maps = make_in_maps_v4(h_t, ch_sorted, *v4)
    else:
        nc = build_nc()
        in_maps = make_in_maps(h_t, ch_sorted, cw_sorted)
    res = run_bass_kernel_spmd(nc, in_maps, core_ids=list(range(NCORES)))
    return np.concatenate([r["vals"].T.reshape(-1) for r in res.results])


def kernel(h_t, cache_h, cache_words):
    h_t, ch_sorted, cw_sorted = _prep(h_t, cache_h, cache_words)
    vals_sorted = run_device(h_t, ch_sorted, cw_sorted)
    return finish_on_host(vals_sorted, cw_sorted)


# revision 11
# speedup vs baseline: 2.0372x; 1.2900x over previous
"""Trainium2 Bass kernel for nn_Cache_28071906246843 (retrieval_knn).

reference semantics:
    q = h_t[cache_words]                         # [C, D] gather
    dist = sqrt(sum((cache_h - q)**2, -1))       # [C]
    vals = exp(dist / 32.0)                      # [C]
    cache_p = segment_sum(vals, cache_words, V)  # [V]
    out = log_softmax(cache_p[None, :])          # [1, V]

Sharding: cache elements are sorted by word id (pure reordering, improves
gather locality) and split into 8 contiguous shards of 16384 elements, one
per NeuronCore.  Each core streams its cache_h shard tile-by-tile
([128, 1024] tiles), gathers the matching h_t rows with an indirect DMA,
and computes the per-element kernel value exp(||h-q||/32) on device
(DVE subtract, ACT square+accumulate, sqrt, exp).  The [V] segment-sum of
8*16384 scalars and the final log_softmax over [V] are done on the host.
"""

import sys

import numpy as np

if "/opt/trn_rl_repo" not in sys.path:
    sys.path.insert(0, "/opt/trn_rl_repo")

import concourse.bass as bass
import concourse.tile as tile
from concourse import bacc, mybir
from concourse.bass_utils import run_bass_kernel_spmd

V, D, C = 50257, 1024, 131072
NCORES = 8
CSH = C // NCORES  # 16384 elements per core
P = 128            # SBUF partitions
NT = CSH // P      # 128 tiles per core
SMOOTH = 32.0


def build_nc(nt: int = NT, v: int = V, d: int = D) -> bass.Bass:
    """Build the per-core Bass program (SPMD: same program on all cores)."""
    nc = bacc.Bacc(
        "TRN2", target_bir_lowering=False, debug=False, num_devices=NCORES
    )
    ht = nc.dram_tensor("ht", [v, d], mybir.dt.float32, kind="ExternalInput")
    ch = nc.dram_tensor("ch", [nt * P, d], mybir.dt.float32, kind="ExternalInput")
    # cw is pre-transposed on host: cw[p, t] = word id of element t*128+p
    cw = nc.dram_tensor("cw", [P, nt], mybir.dt.int32, kind="ExternalInput")
    vals = nc.dram_tensor("vals", [P, nt], mybir.dt.float32, kind="ExternalOutput")

    ch_t = ch.ap().rearrange("(t p) d -> t p d", p=P)  # [nt, 128, d]

    with tile.TileContext(nc) as tc:
        with (
            tc.tile_pool(name="io", bufs=6) as io,
            tc.tile_pool(name="scratch", bufs=2) as scratch,
            tc.tile_pool(name="persist", bufs=1) as persist,
        ):
            cw_sb = persist.tile([P, nt], mybir.dt.int32)
            nc.sync.dma_start(out=cw_sb[:], in_=cw.ap())
            vals_sb = persist.tile([P, nt], mybir.dt.float32)
            d2_all = persist.tile([P, nt], mybir.dt.float32)

            for t in range(nt):
                ch_tile = io.tile([P, d], mybir.dt.float32, tag="ch")
                nc.sync.dma_start(out=ch_tile[:], in_=ch_t[t])

                q_tile = io.tile([P, d], mybir.dt.float32, tag="q")
                nc.gpsimd.indirect_dma_start(
                    out=q_tile[:],
                    out_offset=None,
                    in_=ht.ap(),
                    in_offset=bass.IndirectOffsetOnAxis(
                        ap=cw_sb[:, t : t + 1], axis=0
                    ),
                )

                d_tile = io.tile([P, d], mybir.dt.float32, tag="d")
                nc.vector.tensor_tensor(
                    out=d_tile[:],
                    in0=ch_tile[:],
                    in1=q_tile[:],
                    op=mybir.AluOpType.subtract,
                )

                sq_tile = scratch.tile([P, d], mybir.dt.float32, tag="sq")
                nc.scalar.activation(
                    out=sq_tile[:],
                    in_=d_tile[:],
                    func=mybir.ActivationFunctionType.Square,
                    accum_out=d2_all[:, t : t + 1],
                )

            # batched tail: one table switch each instead of two per tile
            dist_all = persist.tile([P, nt], mybir.dt.float32)
            nc.scalar.activation(
                out=dist_all[:],
                in_=d2_all[:],
                func=mybir.ActivationFunctionType.Sqrt,
            )
            nc.scalar.activation(
                out=vals_sb[:],
                in_=dist_all[:],
                func=mybir.ActivationFunctionType.Exp,
                scale=1.0 / SMOOTH,
            )

            nc.sync.dma_start(out=vals.ap(), in_=vals_sb[:])
    nc.compile()
    return nc


SUP = 2            # element-tiles per supertile
NSUP = NT // SUP   # 64 supertiles per core
SUPW = SUP * P     # 256 elements per supertile


def build_nc_v4(nt: int = NT, v: int = V, d: int = D) -> bass.Bass:
    """Dedup-gather variant: per supertile (256 sorted elements) gather the
    <=128 distinct h_t rows once (cast to bf16 in the DMA), then expand to
    per-element rows with a one-hot matmul on the TensorEngine
    (Q = S^T @ W).  The one-hot S comes precomputed from the host."""
    nsup = nt // SUP
    nc = bacc.Bacc(
        "TRN2", target_bir_lowering=False, debug=False, num_devices=NCORES
    )
    ht = nc.dram_tensor("ht", [v, d], mybir.dt.float32, kind="ExternalInput")
    ch = nc.dram_tensor("ch", [nt * P, d], mybir.dt.float32, kind="ExternalInput")
    # widx[p, s] = p-th (padded) distinct word id of supertile s
    widx = nc.dram_tensor("widx", [P, nsup], mybir.dt.int32, kind="ExternalInput")
    # sel[t, w, e] = 1 iff element t*128+e's word is the w-th distinct word
    # of supertile t//SUP
    sel = nc.dram_tensor("sel", [nt, P, P], mybir.dt.bfloat16, kind="ExternalInput")
    vals = nc.dram_tensor("vals", [P, nt], mybir.dt.float32, kind="ExternalOutput")

    ch_t = ch.ap().rearrange("(t p) d -> t p d", p=P)  # [nt, 128, d]

    with tile.TileContext(nc) as tc:
        with (
            tc.tile_pool(name="io", bufs=6) as io,
            tc.tile_pool(name="wpool", bufs=3) as wpool,
            tc.tile_pool(name="spool", bufs=6) as spool,
            tc.tile_pool(name="psum", bufs=2, space="PSUM") as psum,
            tc.tile_pool(name="scratch", bufs=2) as scratch,
            tc.tile_pool(name="persist", bufs=1) as persist,
        ):
            widx_sb = persist.tile([P, nsup], mybir.dt.int32)
            nc.sync.dma_start(out=widx_sb[:], in_=widx.ap())
            vals_sb = persist.tile([P, nt], mybir.dt.float32)
            d2_all = persist.tile([P, nt], mybir.dt.float32)

            for s in range(nsup):
                w_bf = wpool.tile([P, d], mybir.dt.bfloat16, tag="wbf")
                nc.gpsimd.indirect_dma_start(
                    out=w_bf[:],
                    out_offset=None,
                    in_=ht.ap(),
                    in_offset=bass.IndirectOffsetOnAxis(
                        ap=widx_sb[:, s : s + 1], axis=0
                    ),
                )

                for k in range(SUP):
                    t = SUP * s + k
                    ch_tile = io.tile([P, d], mybir.dt.float32, tag="ch")
                    nc.sync.dma_start(out=ch_tile[:], in_=ch_t[t])

                    s_onehot = spool.tile([P, P], mybir.dt.bfloat16, tag="sel")
                    nc.sync.dma_start(out=s_onehot[:], in_=sel.ap()[t])

                    q_psum = psum.tile([P, d], mybir.dt.float32)
                    for h in range(0, d, 512):
                        nc.tensor.matmul(
                            out=q_psum[:, h : h + 512],
                            lhsT=s_onehot[:],
                            rhs=w_bf[:, h : h + 512],
                            start=True,
                            stop=True,
                        )

                    d_tile = io.tile([P, d], mybir.dt.float32, tag="d")
                    nc.vector.tensor_tensor(
                        out=d_tile[:],
                        in0=ch_tile[:],
                        in1=q_psum[:],
                        op=mybir.AluOpType.subtract,
                    )

                    sq_tile = scratch.tile([P, d], mybir.dt.float32, tag="sq")
                    nc.scalar.activation(
                        out=sq_tile[:],
                        in_=d_tile[:],
                        func=mybir.ActivationFunctionType.Square,
                        accum_out=d2_all[:, t : t + 1],
                    )

            dist_all = persist.tile([P, nt], mybir.dt.float32)
            nc.scalar.activation(
                out=dist_all[:],
                in_=d2_all[:],
                func=mybir.ActivationFunctionType.Sqrt,
            )
            nc.scalar.activation(
                out=vals_sb[:],
                in_=dist_all[:],
                func=mybir.ActivationFunctionType.Exp,
                scale=1.0 / SMOOTH,
            )
            nc.sync.dma_start(out=vals.ap(), in_=vals_sb[:])
    nc.compile()
    return nc


def prep_v4(cw_sorted):
    """Per-core supertile metadata. Returns None if any supertile has more
    than 128 distinct words (fall back to per-element gather then)."""
    import ml_dtypes

    widx_all, sel_all = [], []
    eye = np.eye(P, dtype=ml_dtypes.bfloat16)
    for c in range(NCORES):
        shard = cw_sorted[c * CSH : (c + 1) * CSH]
        widx = np.empty((NSUP, P), np.int32)
        sel = np.empty((NT, P, P), ml_dtypes.bfloat16)
        for s in range(NSUP):
            seg = shard[s * SUPW : (s + 1) * SUPW]
            uw = np.unique(seg)
            if len(uw) > P:
                return None
            widx[s, : len(uw)] = uw
            widx[s, len(uw) :] = uw[-1]
            rel = np.searchsorted(uw, seg).reshape(SUP, P)
            for k in range(SUP):
                # sel[t][w, e] = 1 iff rel[k][e] == w
                sel[SUP * s + k] = eye[:, rel[k]]
        widx_all.append(np.ascontiguousarray(widx.T))
        sel_all.append(sel)
    return widx_all, sel_all


def make_in_maps_v4(h_t, ch_sorted, widx_all, sel_all):
    in_maps = []
    for c in range(NCORES):
        sl = slice(c * CSH, (c + 1) * CSH)
        in_maps.append(
            {
                "ht": h_t,
                "ch": ch_sorted[sl],
                "widx": widx_all[c],
                "sel": sel_all[c],
            }
        )
    return in_maps


def make_in_maps(h_t, ch_sorted, cw_sorted):
    in_maps = []
    for c in range(NCORES):
        sl = slice(c * CSH, (c + 1) * CSH)
        in_maps.append(
            {
                "ht": h_t,
                "ch": ch_sorted[sl],
                "cw": np.ascontiguousarray(cw_sorted[sl].reshape(NT, P).T),
            }
        )
    return in_maps


def finish_on_host(vals_sorted, cw_sorted):
    """segment-sum + log_softmax (tiny O(C)+O(V) work)."""
    p = np.bincount(cw_sorted, weights=vals_sorted.astype(np.float64), minlength=V)
    m = p.max()
    lse = m + np.log(np.exp(p - m).sum())
    return (p - lse).astype(np.float32)[None, :]


def _prep(h_t, cache_h, cache_words):
    h_t = np.ascontiguousarray(np.asarray(h_t), dtype=np.float32)
    cache_h = np.ascontiguousarray(np.asarray(cache_h), dtype=np.float32)
    cw = np.asarray(cache_words).astype(np.int32)
    order = np.argsort(cw, kind="stable")
    return h_t, cache_h[order], cw[order]


def run_device(h_t, ch_sorted, cw_sorted, force_v1=False):
    """Compile + run the SPMD program; returns per-element vals (sorted order)."""
    v4 = None if force_v1 else prep_v4(cw_sorted)
    if v4 is not None:
        nc = build_nc_v4()
        in_maps = make_in_maps_v4(h_t, ch_sorted, *v4)
    else:
        nc = build_nc()
        in_maps = make_in_maps(h_t, ch_sorted, cw_sorted)
    res = run_bass_kernel_spmd(nc, in_maps, core_ids=list(range(NCORES)))
    return np.concatenate([r["vals"].T.reshape(-1) for r in res.results])


def kernel(h_t, cache_h, cache_words):
    h_t, ch_sorted, cw_sorted = _prep(h_t, cache_h, cache_words)
    vals_sorted = run_device(h_t, ch_sorted, cw_sorted)
    return finish_on_host(vals_sorted, cw_sorted)


# revision 14
# speedup vs baseline: 2.1591x; 1.0599x over previous
"""Trainium2 Bass kernel for nn_Cache_28071906246843 (retrieval_knn).

reference semantics:
    q = h_t[cache_words]                         # [C, D] gather
    dist = sqrt(sum((cache_h - q)**2, -1))       # [C]
    vals = exp(dist / 32.0)                      # [C]
    cache_p = segment_sum(vals, cache_words, V)  # [V]
    out = log_softmax(cache_p[None, :])          # [1, V]

Sharding: cache elements are sorted by word id (pure reordering, improves
gather locality) and split into 8 contiguous shards of 16384 elements, one
per NeuronCore.  Each core streams its cache_h shard tile-by-tile
([128, 1024] tiles), gathers the matching h_t rows with an indirect DMA,
and computes the per-element kernel value exp(||h-q||/32) on device
(DVE subtract, ACT square+accumulate, sqrt, exp).  The [V] segment-sum of
8*16384 scalars and the final log_softmax over [V] are done on the host.
"""

import sys

import numpy as np

if "/opt/trn_rl_repo" not in sys.path:
    sys.path.insert(0, "/opt/trn_rl_repo")

import concourse.bass as bass
import concourse.tile as tile
from concourse import bacc, mybir
from concourse.bass_utils import run_bass_kernel_spmd

V, D, C = 50257, 1024, 131072
NCORES = 8
CSH = C // NCORES  # 16384 elements per core
P = 128            # SBUF partitions
NT = CSH // P      # 128 tiles per core
SMOOTH = 32.0


def build_nc(nt: int = NT, v: int = V, d: int = D) -> bass.Bass:
    """Build the per-core Bass program (SPMD: same program on all cores)."""
    nc = bacc.Bacc(
        "TRN2", target_bir_lowering=False, debug=False, num_devices=NCORES
    )
    ht = nc.dram_tensor("ht", [v, d], mybir.dt.float32, kind="ExternalInput")
    ch = nc.dram_tensor("ch", [nt * P, d], mybir.dt.float32, kind="ExternalInput")
    # cw is pre-transposed on host: cw[p, t] = word id of element t*128+p
    cw = nc.dram_tensor("cw", [P, nt], mybir.dt.int32, kind="ExternalInput")
    vals = nc.dram_tensor("vals", [P, nt], mybir.dt.float32, kind="ExternalOutput")

    ch_t = ch.ap().rearrange("(t p) d -> t p d", p=P)  # [nt, 128, d]

    with tile.TileContext(nc) as tc:
        with (
            tc.tile_pool(name="io", bufs=6) as io,
            tc.tile_pool(name="scratch", bufs=2) as scratch,
            tc.tile_pool(name="persist", bufs=1) as persist,
        ):
            cw_sb = persist.tile([P, nt], mybir.dt.int32)
            nc.sync.dma_start(out=cw_sb[:], in_=cw.ap())
            vals_sb = persist.tile([P, nt], mybir.dt.float32)
            d2_all = persist.tile([P, nt], mybir.dt.float32)

            for t in range(nt):
                ch_tile = io.tile([P, d], mybir.dt.float32, tag="ch")
                nc.sync.dma_start(out=ch_tile[:], in_=ch_t[t])

                q_tile = io.tile([P, d], mybir.dt.float32, tag="q")
                nc.gpsimd.indirect_dma_start(
                    out=q_tile[:],
                    out_offset=None,
                    in_=ht.ap(),
                    in_offset=bass.IndirectOffsetOnAxis(
                        ap=cw_sb[:, t : t + 1], axis=0
                    ),
                )

                d_tile = io.tile([P, d], mybir.dt.float32, tag="d")
                nc.vector.tensor_tensor(
                    out=d_tile[:],
                    in0=ch_tile[:],
                    in1=q_tile[:],
                    op=mybir.AluOpType.subtract,
                )

                sq_tile = scratch.tile([P, d], mybir.dt.float32, tag="sq")
                nc.scalar.activation(
                    out=sq_tile[:],
                    in_=d_tile[:],
                    func=mybir.ActivationFunctionType.Square,
                    accum_out=d2_all[:, t : t + 1],
                )

            # batched tail: one table switch each instead of two per tile
            dist_all = persist.tile([P, nt], mybir.dt.float32)
            nc.scalar.activation(
                out=dist_all[:],
                in_=d2_all[:],
                func=mybir.ActivationFunctionType.Sqrt,
            )
            nc.scalar.activation(
                out=vals_sb[:],
                in_=dist_all[:],
                func=mybir.ActivationFunctionType.Exp,
                scale=1.0 / SMOOTH,
            )

            nc.sync.dma_start(out=vals.ap(), in_=vals_sb[:])
    nc.compile()
    return nc


SUP = 2            # element-tiles per supertile
NSUP = NT // SUP   # 64 supertiles per core
SUPW = SUP * P     # 256 elements per supertile


def build_nc_v4(nt: int = NT, v: int = V, d: int = D) -> bass.Bass:
    """Dedup-gather variant.

    Per supertile (256 sorted elements): gather the <=128 distinct h_t rows
    once (cast to fp16 during the DMA).  Per element-tile, the TensorEngine
    computes d = I^T @ ch - S^T @ W directly in PSUM (host supplies the
    NEGATED one-hot S and the identity), so VectorE is not used at all.
    ScalarE squares+accumulates straight from PSUM; sqrt/exp run batched at
    the end.  ch arrives as fp16 (host cast) to halve its HBM traffic."""
    nsup = nt // SUP
    nc = bacc.Bacc(
        "TRN2", target_bir_lowering=False, debug=False, num_devices=NCORES
    )
    ht = nc.dram_tensor("ht", [v, d], mybir.dt.float32, kind="ExternalInput")
    ch = nc.dram_tensor("ch", [nt * P, d], mybir.dt.float16, kind="ExternalInput")
    # widx[p, s] = p-th (padded) distinct word id of supertile s
    widx = nc.dram_tensor("widx", [P, nsup], mybir.dt.int32, kind="ExternalInput")
    # nsel[t, w, e] = -1 iff element t*128+e's word is the w-th distinct word
    # of supertile t//SUP, else 0
    nsel = nc.dram_tensor("nsel", [nt, P, P], mybir.dt.float16, kind="ExternalInput")
    ident = nc.dram_tensor("ident", [P, P], mybir.dt.float16, kind="ExternalInput")
    vals = nc.dram_tensor("vals", [P, nt], mybir.dt.float32, kind="ExternalOutput")

    ch_ap = ch.ap()      # [nt*P, d]
    nsel_ap = nsel.ap()  # [nt, P, P]

    with tile.TileContext(nc) as tc:
        with (
            tc.tile_pool(name="io", bufs=4) as io,
            tc.tile_pool(name="wpool", bufs=3) as wpool,
            tc.tile_pool(name="spool", bufs=4) as spool,
            tc.tile_pool(name="psum", bufs=3, space="PSUM") as psum,
            tc.tile_pool(name="scratch", bufs=2) as scratch,
            tc.tile_pool(name="persist", bufs=1) as persist,
        ):
            widx_sb = persist.tile([P, nsup], mybir.dt.int32)
            nc.sync.dma_start(out=widx_sb[:], in_=widx.ap())
            ident_sb = persist.tile([P, P], mybir.dt.float16)
            nc.sync.dma_start(out=ident_sb[:], in_=ident.ap())
            vals_sb = persist.tile([P, nt], mybir.dt.float32)
            d2_all = persist.tile([P, nt], mybir.dt.float32)

            for s in range(nsup):
                w_fp = wpool.tile([P, d], mybir.dt.float16, tag="wfp")
                nc.gpsimd.indirect_dma_start(
                    out=w_fp[:],
                    out_offset=None,
                    in_=ht.ap(),
                    in_offset=bass.IndirectOffsetOnAxis(
                        ap=widx_sb[:, s : s + 1], axis=0
                    ),
                )

                # one DMA per supertile for ch ([128, SUP, d]) and nsel
                # ([128, SUP, 128])
                ch_sb = io.tile([P, SUP, d], mybir.dt.float16, tag="ch")
                ch_src = bass.AP(
                    tensor=ch_ap.tensor,
                    offset=s * SUPW * d,
                    ap=[[d, P], [P * d, SUP], [1, d]],
                )
                nc.sync.dma_start(out=ch_sb[:], in_=ch_src)

                ns_sb = spool.tile([P, SUP, P], mybir.dt.float16, tag="nsel")
                ns_src = bass.AP(
                    tensor=nsel_ap.tensor,
                    offset=s * SUP * P * P,
                    ap=[[P, P], [P * P, SUP], [1, P]],
                )
                nc.sync.dma_start(out=ns_sb[:], in_=ns_src)

                for k in range(SUP):
                    t = SUP * s + k
                    q_psum = psum.tile([P, d], mybir.dt.float32)
                    for h in range(0, d, 512):
                        nc.tensor.matmul(
                            out=q_psum[:, h : h + 512],
                            lhsT=ns_sb[:, k, :],
                            rhs=w_fp[:, h : h + 512],
                            start=True,
                            stop=False,
                        )
                    for h in range(0, d, 512):
                        nc.tensor.matmul(
                            out=q_psum[:, h : h + 512],
                            lhsT=ident_sb[:],
                            rhs=ch_sb[:, k, h : h + 512],
                            start=False,
                            stop=True,
                        )

                    sq_tile = scratch.tile([P, d], mybir.dt.float32, tag="sq")
                    nc.scalar.activation(
                        out=sq_tile[:],
                        in_=q_psum[:],
                        func=mybir.ActivationFunctionType.Square,
                        accum_out=d2_all[:, t : t + 1],
                    )

            dist_all = persist.tile([P, nt], mybir.dt.float32)
            nc.scalar.activation(
                out=dist_all[:],
                in_=d2_all[:],
                func=mybir.ActivationFunctionType.Sqrt,
            )
            nc.scalar.activation(
                out=vals_sb[:],
                in_=dist_all[:],
                func=mybir.ActivationFunctionType.Exp,
                scale=1.0 / SMOOTH,
            )
            nc.sync.dma_start(out=vals.ap(), in_=vals_sb[:])
    nc.compile()
    return nc


def prep_v4(cw_sorted):
    """Per-core supertile metadata. Returns None if any supertile has more
    than 128 distinct words (fall back to per-element gather then)."""
    widx_all, nsel_all = [], []
    neye = -np.eye(P, dtype=np.float16)
    for c in range(NCORES):
        shard = cw_sorted[c * CSH : (c + 1) * CSH]
        widx = np.empty((NSUP, P), np.int32)
        nsel = np.empty((NT, P, P), np.float16)
        for s in range(NSUP):
            seg = shard[s * SUPW : (s + 1) * SUPW]
            uw = np.unique(seg)
            if len(uw) > P:
                return None
            widx[s, : len(uw)] = uw
            widx[s, len(uw) :] = uw[-1]
            rel = np.searchsorted(uw, seg).reshape(SUP, P)
            for k in range(SUP):
                # nsel[t][w, e] = -1 iff rel[k][e] == w
                nsel[SUP * s + k] = neye[:, rel[k]]
        widx_all.append(np.ascontiguousarray(widx.T))
        nsel_all.append(nsel)
    return widx_all, nsel_all


def make_in_maps_v4(h_t, ch_sorted, widx_all, nsel_all):
    ident = np.eye(P, dtype=np.float16)
    in_maps = []
    for c in range(NCORES):
        sl = slice(c * CSH, (c + 1) * CSH)
        in_maps.append(
            {
                "ht": h_t,
                "ch": ch_sorted[sl].astype(np.float16),
                "widx": widx_all[c],
                "nsel": nsel_all[c],
                "ident": ident,
            }
        )
    return in_maps


def make_in_maps(h_t, ch_sorted, cw_sorted):
    in_maps = []
    for c in range(NCORES):
        sl = slice(c * CSH, (c + 1) * CSH)
        in_maps.append(
            {
                "ht": h_t,
                "ch": ch_sorted[sl],
                "cw": np.ascontiguousarray(cw_sorted[sl].reshape(NT, P).T),
            }
        )
    return in_maps


def finish_on_host(vals_sorted, cw_sorted):
    """segment-sum + log_softmax (tiny O(C)+O(V) work)."""
    p = np.bincount(cw_sorted, weights=vals_sorted.astype(np.float64), minlength=V)
    m = p.max()
    lse = m + np.log(np.exp(p - m).sum())
    return (p - lse).astype(np.float32)[None, :]


def _prep(h_t, cache_h, cache_words):
    h_t = np.ascontiguousarray(np.asarray(h_t), dtype=np.float32)
    cache_h = np.ascontiguousarray(np.asarray(cache_h), dtype=np.float32)
    cw = np.asarray(cache_words).astype(np.int32)
    order = np.argsort(cw, kind="stable")
    return h_t, cache_h[order], cw[order]


def run_device(h_t, ch_sorted, cw_sorted, force_v1=False, verbose=False):
    """Compile + run the SPMD program; returns per-element vals (sorted order)."""
    import time as _time

    _t0 = _time.time()
    v4 = None if force_v1 else prep_v4(cw_sorted)
    if v4 is not None:
        nc = build_nc_v4()
        in_maps = make_in_maps_v4(h_t, ch_sorted, *v4)
    else:
        nc = build_nc()
        in_maps = make_in_maps(h_t, ch_sorted, cw_sorted)
    if verbose:
        print(f"[run_device] build+prep: {_time.time() - _t0:.1f}s")
    _t1 = _time.time()
    res = run_bass_kernel_spmd(nc, in_maps, core_ids=list(range(NCORES)))
    if verbose:
        print(f"[run_device] compile+exec: {_time.time() - _t1:.1f}s")
    return np.concatenate([r["vals"].T.reshape(-1) for r in res.results])


def kernel(h_t, cache_h, cache_words):
    h_t, ch_sorted, cw_sorted = _prep(h_t, cache_h, cache_words)
    vals_sorted = run_device(h_t, ch_sorted, cw_sorted)
    return finish_on_host(vals_sorted, cw_sorted)


# revision 17
# speedup vs baseline: 2.3979x; 1.1106x over previous
"""Trainium2 Bass kernel for nn_Cache_28071906246843 (retrieval_knn).

reference semantics:
    q = h_t[cache_words]                         # [C, D] gather
    dist = sqrt(sum((cache_h - q)**2, -1))       # [C]
    vals = exp(dist / 32.0)                      # [C]
    cache_p = segment_sum(vals, cache_words, V)  # [V]
    out = log_softmax(cache_p[None, :])          # [1, V]

Sharding: cache elements are sorted by word id (pure reordering, improves
gather locality) and split into 8 contiguous shards of 16384 elements, one
per NeuronCore.  Each core streams its cache_h shard tile-by-tile
([128, 1024] tiles), gathers the matching h_t rows with an indirect DMA,
and computes the per-element kernel value exp(||h-q||/32) on device
(DVE subtract, ACT square+accumulate, sqrt, exp).  The [V] segment-sum of
8*16384 scalars and the final log_softmax over [V] are done on the host.
"""

import sys

import numpy as np

if "/opt/trn_rl_repo" not in sys.path:
    sys.path.insert(0, "/opt/trn_rl_repo")

import concourse.bass as bass
import concourse.tile as tile
from concourse import bacc, mybir
from concourse.bass_utils import run_bass_kernel_spmd

V, D, C = 50257, 1024, 131072
NCORES = 8
CSH = C // NCORES  # 16384 elements per core
P = 128            # SBUF partitions
NT = CSH // P      # 128 tiles per core
SMOOTH = 32.0


def build_nc(nt: int = NT, v: int = V, d: int = D) -> bass.Bass:
    """Build the per-core Bass program (SPMD: same program on all cores)."""
    nc = bacc.Bacc(
        "TRN2", target_bir_lowering=False, debug=False, num_devices=NCORES
    )
    ht = nc.dram_tensor("ht", [v, d], mybir.dt.float32, kind="ExternalInput")
    ch = nc.dram_tensor("ch", [nt * P, d], mybir.dt.float32, kind="ExternalInput")
    # cw is pre-transposed on host: cw[p, t] = word id of element t*128+p
    cw = nc.dram_tensor("cw", [P, nt], mybir.dt.int32, kind="ExternalInput")
    vals = nc.dram_tensor("vals", [P, nt], mybir.dt.float32, kind="ExternalOutput")

    ch_t = ch.ap().rearrange("(t p) d -> t p d", p=P)  # [nt, 128, d]

    with tile.TileContext(nc) as tc:
        with (
            tc.tile_pool(name="io", bufs=6) as io,
            tc.tile_pool(name="scratch", bufs=2) as scratch,
            tc.tile_pool(name="persist", bufs=1) as persist,
        ):
            cw_sb = persist.tile([P, nt], mybir.dt.int32)
            nc.sync.dma_start(out=cw_sb[:], in_=cw.ap())
            vals_sb = persist.tile([P, nt], mybir.dt.float32)
            d2_all = persist.tile([P, nt], mybir.dt.float32)

            for t in range(nt):
                ch_tile = io.tile([P, d], mybir.dt.float32, tag="ch")
                nc.sync.dma_start(out=ch_tile[:], in_=ch_t[t])

                q_tile = io.tile([P, d], mybir.dt.float32, tag="q")
                nc.gpsimd.indirect_dma_start(
                    out=q_tile[:],
                    out_offset=None,
                    in_=ht.ap(),
                    in_offset=bass.IndirectOffsetOnAxis(
                        ap=cw_sb[:, t : t + 1], axis=0
                    ),
                )

                d_tile = io.tile([P, d], mybir.dt.float32, tag="d")
                nc.vector.tensor_tensor(
                    out=d_tile[:],
                    in0=ch_tile[:],
                    in1=q_tile[:],
                    op=mybir.AluOpType.subtract,
                )

                sq_tile = scratch.tile([P, d], mybir.dt.float32, tag="sq")
                nc.scalar.activation(
                    out=sq_tile[:],
                    in_=d_tile[:],
                    func=mybir.ActivationFunctionType.Square,
                    accum_out=d2_all[:, t : t + 1],
                )

            # batched tail: one table switch each instead of two per tile
            dist_all = persist.tile([P, nt], mybir.dt.float32)
            nc.scalar.activation(
                out=dist_all[:],
                in_=d2_all[:],
                func=mybir.ActivationFunctionType.Sqrt,
            )
            nc.scalar.activation(
                out=vals_sb[:],
                in_=dist_all[:],
                func=mybir.ActivationFunctionType.Exp,
                scale=1.0 / SMOOTH,
            )

            nc.sync.dma_start(out=vals.ap(), in_=vals_sb[:])
    nc.compile()
    return nc


SUP = 2            # element-tiles per supertile
NSUP = NT // SUP   # 64 supertiles per core
SUPW = SUP * P     # 256 elements per supertile


def build_nc_v4(nt: int = NT, v: int = V, d: int = D) -> bass.Bass:
    """Dedup-gather variant.

    Per supertile (256 sorted elements): gather the <=128 distinct h_t rows
    once (cast to fp16 during the DMA).  Per element-tile, the TensorEngine
    computes d = I^T @ ch - S^T @ W directly in PSUM (host supplies the
    NEGATED one-hot S and the identity), so VectorE is not used at all.
    ScalarE squares+accumulates straight from PSUM; sqrt/exp run batched at
    the end.  ch arrives as fp16 (host cast) to halve its HBM traffic."""
    nsup = nt // SUP
    nc = bacc.Bacc(
        "TRN2", target_bir_lowering=False, debug=False, num_devices=NCORES
    )
    ht = nc.dram_tensor("ht", [v, d], mybir.dt.float32, kind="ExternalInput")
    ch = nc.dram_tensor("ch", [nt * P, d], mybir.dt.float16, kind="ExternalInput")
    # widx[p, s] = p-th (padded) distinct word id of supertile s
    widx = nc.dram_tensor("widx", [P, nsup], mybir.dt.int32, kind="ExternalInput")
    # nsel[t, w, e] = -1 iff element t*128+e's word is the w-th distinct word
    # of supertile t//SUP, else 0
    nsel = nc.dram_tensor("nsel", [nt, P, P], mybir.dt.float16, kind="ExternalInput")
    ident = nc.dram_tensor("ident", [P, P], mybir.dt.float16, kind="ExternalInput")
    vals = nc.dram_tensor("vals", [P, nt], mybir.dt.float32, kind="ExternalOutput")

    ch_ap = ch.ap()      # [nt*P, d]
    nsel_ap = nsel.ap()  # [nt, P, P]

    with tile.TileContext(nc) as tc:
        with (
            tc.tile_pool(name="io", bufs=4) as io,
            tc.tile_pool(name="wpool", bufs=3) as wpool,
            tc.tile_pool(name="spool", bufs=4) as spool,
            tc.tile_pool(name="psum", bufs=4, space="PSUM") as psum,
            tc.tile_pool(name="scratch", bufs=2) as scratch,
            tc.tile_pool(name="persist", bufs=1) as persist,
        ):
            widx_sb = persist.tile([P, nsup], mybir.dt.int32)
            nc.sync.dma_start(out=widx_sb[:], in_=widx.ap())
            ident_sb = persist.tile([P, P], mybir.dt.float16)
            nc.sync.dma_start(out=ident_sb[:], in_=ident.ap())
            vals_sb = persist.tile([P, nt], mybir.dt.float32)
            d2_all = persist.tile([P, nt], mybir.dt.float32)

            for s in range(nsup):
                w_fp = wpool.tile([P, d], mybir.dt.float16, tag="wfp")
                nc.gpsimd.indirect_dma_start(
                    out=w_fp[:],
                    out_offset=None,
                    in_=ht.ap(),
                    in_offset=bass.IndirectOffsetOnAxis(
                        ap=widx_sb[:, s : s + 1], axis=0
                    ),
                )

                # one DMA per supertile for ch ([128, SUP, d]) and nsel
                # ([128, SUP, 128])
                ch_sb = io.tile([P, SUP, d], mybir.dt.float16, tag="ch")
                ch_src = bass.AP(
                    tensor=ch_ap.tensor,
                    offset=s * SUPW * d,
                    ap=[[d, P], [P * d, SUP], [1, d]],
                )
                nc.sync.dma_start(out=ch_sb[:], in_=ch_src)

                ns_sb = spool.tile([P, SUP, P], mybir.dt.float16, tag="nsel")
                ns_src = bass.AP(
                    tensor=nsel_ap.tensor,
                    offset=s * SUP * P * P,
                    ap=[[P, P], [P * P, SUP], [1, P]],
                )
                nc.sync.dma_start(out=ns_sb[:], in_=ns_src)

                # order matmuls so each lhsT is loaded once per supertile:
                # ns0 (2 halves), ns1 (2 halves), then identity (4 halves)
                q_psums = []
                for k in range(SUP):
                    q_psum = psum.tile([P, d], mybir.dt.float32, tag="q")
                    q_psums.append(q_psum)
                    for h in range(0, d, 512):
                        nc.tensor.matmul(
                            out=q_psum[:, h : h + 512],
                            lhsT=ns_sb[:, k, :],
                            rhs=w_fp[:, h : h + 512],
                            start=True,
                            stop=False,
                        )
                for k in range(SUP):
                    for h in range(0, d, 512):
                        nc.tensor.matmul(
                            out=q_psums[k][:, h : h + 512],
                            lhsT=ident_sb[:],
                            rhs=ch_sb[:, k, h : h + 512],
                            start=False,
                            stop=True,
                        )
                for k in range(SUP):
                    t = SUP * s + k
                    sq_tile = scratch.tile([P, d], mybir.dt.float32, tag="sq")
                    nc.scalar.activation(
                        out=sq_tile[:],
                        in_=q_psums[k][:],
                        func=mybir.ActivationFunctionType.Square,
                        accum_out=d2_all[:, t : t + 1],
                    )

            dist_all = persist.tile([P, nt], mybir.dt.float32)
            nc.scalar.activation(
                out=dist_all[:],
                in_=d2_all[:],
                func=mybir.ActivationFunctionType.Sqrt,
            )
            nc.scalar.activation(
                out=vals_sb[:],
                in_=dist_all[:],
                func=mybir.ActivationFunctionType.Exp,
                scale=1.0 / SMOOTH,
            )
            nc.sync.dma_start(out=vals.ap(), in_=vals_sb[:])
    nc.compile()
    return nc


def prep_v4(cw_sorted):
    """Per-core supertile metadata. Returns None if any supertile has more
    than 128 distinct words (fall back to per-element gather then)."""
    widx_all, nsel_all = [], []
    neye = -np.eye(P, dtype=np.float16)
    for c in range(NCORES):
        shard = cw_sorted[c * CSH : (c + 1) * CSH]
        widx = np.empty((NSUP, P), np.int32)
        nsel = np.empty((NT, P, P), np.float16)
        for s in range(NSUP):
            seg = shard[s * SUPW : (s + 1) * SUPW]
            uw = np.unique(seg)
            if len(uw) > P:
                return None
            widx[s, : len(uw)] = uw
            widx[s, len(uw) :] = uw[-1]
            rel = np.searchsorted(uw, seg).reshape(SUP, P)
            for k in range(SUP):
                # nsel[t][w, e] = -1 iff rel[k][e] == w
                nsel[SUP * s + k] = neye[:, rel[k]]
        widx_all.append(np.ascontiguousarray(widx.T))
        nsel_all.append(nsel)
    return widx_all, nsel_all


def make_in_maps_v4(h_t, ch_sorted, widx_all, nsel_all):
    ident = np.eye(P, dtype=np.float16)
    in_maps = []
    for c in range(NCORES):
        sl = slice(c * CSH, (c + 1) * CSH)
        in_maps.append(
            {
                "ht": h_t,
                "ch": ch_sorted[sl].astype(np.float16),
                "widx": widx_all[c],
                "nsel": nsel_all[c],
                "ident": ident,
            }
        )
    return in_maps


def make_in_maps(h_t, ch_sorted, cw_sorted):
    in_maps = []
    for c in range(NCORES):
        sl = slice(c * CSH, (c + 1) * CSH)
        in_maps.append(
            {
                "ht": h_t,
                "ch": ch_sorted[sl],
                "cw": np.ascontiguousarray(cw_sorted[sl].reshape(NT, P).T),
            }
        )
    return in_maps


def finish_on_host(vals_sorted, cw_sorted):
    """segment-sum + log_softmax (tiny O(C)+O(V) work)."""
    p = np.bincount(cw_sorted, weights=vals_sorted.astype(np.float64), minlength=V)
    m = p.max()
    lse = m + np.log(np.exp(p - m).sum())
    return (p - lse).astype(np.float32)[None, :]


def _prep(h_t, cache_h, cache_words):
    h_t = np.ascontiguousarray(np.asarray(h_t), dtype=np.float32)
    cache_h = np.ascontiguousarray(np.asarray(cache_h), dtype=np.float32)
    cw = np.asarray(cache_words).astype(np.int32)
    order = np.argsort(cw, kind="stable")
    return h_t, cache_h[order], cw[order]


def run_device(h_t, ch_sorted, cw_sorted, force_v1=False, verbose=False):
    """Compile + run the SPMD program; returns per-element vals (sorted order)."""
    import time as _time

    _t0 = _time.time()
    v4 = None if force_v1 else prep_v4(cw_sorted)
    if v4 is not None:
        nc = build_nc_v4()
        in_maps = make_in_maps_v4(h_t, ch_sorted, *v4)
    else:
        nc = build_nc()
        in_maps = make_in_maps(h_t, ch_sorted, cw_sorted)
    if verbose:
        print(f"[run_device] build+prep: {_time.time() - _t0:.1f}s")
    _t1 = _time.time()
    res = run_bass_kernel_spmd(nc, in_maps, core_ids=list(range(NCORES)))
    if verbose:
        print(f"[run_device] compile+exec: {_time.time() - _t1:.1f}s")
    return np.concatenate([r["vals"].T.reshape(-1) for r in res.results])


def kernel(h_t, cache_h, cache_words):
    h_t, ch_sorted, cw_sorted = _prep(h_t, cache_h, cache_words)
    vals_sorted = run_device(h_t, ch_sorted, cw_sorted)
    return finish_on_host(vals_sorted, cw_sorted)


# revision 23
# speedup vs baseline: 2.6924x; 1.1228x over previous
"""Trainium2 Bass kernel for nn_Cache_28071906246843 (retrieval_knn).

reference semantics:
    q = h_t[cache_words]                         # [C, D] gather
    dist = sqrt(sum((cache_h - q)**2, -1))       # [C]
    vals = exp(dist / 32.0)                      # [C]
    cache_p = segment_sum(vals, cache_words, V)  # [V]
    out = log_softmax(cache_p[None, :])          # [1, V]

Sharding: cache elements are sorted by word id (pure reordering) and split
into 8 contiguous shards of 16384 elements, one per NeuronCore.  Each core
(main path, build_nc_v4): per supertile of 256 sorted elements, indirect-DMA
gathers the <=128 distinct h_t rows once (f32 -> fp16 cast in the DMA),
then the TensorEngine expands them to per-element rows and subtracts in one
step (d = I^T @ ch - S^T @ W accumulated in PSUM, with the negated one-hot
S precomputed on the host from the indices); ScalarE squares+accumulates
straight from PSUM, with batched sqrt/exp at the end.  A per-element-gather
fallback (build_nc) handles the unlikely case of >128 distinct words in a
supertile.  The [V] segment-sum of the 131072 scalars and the final
log_softmax over [V] are done on the host.
"""

import sys

import numpy as np

if "/opt/trn_rl_repo" not in sys.path:
    sys.path.insert(0, "/opt/trn_rl_repo")

import concourse.bass as bass
import concourse.tile as tile
from concourse import bacc, mybir
from concourse.bass_utils import run_bass_kernel_spmd

V, D, C = 50257, 1024, 131072
NCORES = 8
CSH = C // NCORES  # 16384 elements per core
P = 128            # SBUF partitions
NT = CSH // P      # 128 tiles per core
SMOOTH = 32.0


def build_nc(nt: int = NT, v: int = V, d: int = D) -> bass.Bass:
    """Build the per-core Bass program (SPMD: same program on all cores)."""
    nc = bacc.Bacc(
        "TRN2", target_bir_lowering=False, debug=False, num_devices=NCORES
    )
    ht = nc.dram_tensor("ht", [v, d], mybir.dt.float32, kind="ExternalInput")
    ch = nc.dram_tensor("ch", [nt * P, d], mybir.dt.float32, kind="ExternalInput")
    # cw is pre-transposed on host: cw[p, t] = word id of element t*128+p
    cw = nc.dram_tensor("cw", [P, nt], mybir.dt.int32, kind="ExternalInput")
    vals = nc.dram_tensor("vals", [P, nt], mybir.dt.float32, kind="ExternalOutput")

    ch_t = ch.ap().rearrange("(t p) d -> t p d", p=P)  # [nt, 128, d]

    with tile.TileContext(nc) as tc:
        with (
            tc.tile_pool(name="io", bufs=6) as io,
            tc.tile_pool(name="scratch", bufs=2) as scratch,
            tc.tile_pool(name="persist", bufs=1) as persist,
        ):
            cw_sb = persist.tile([P, nt], mybir.dt.int32)
            nc.sync.dma_start(out=cw_sb[:], in_=cw.ap())
            vals_sb = persist.tile([P, nt], mybir.dt.float32)
            d2_all = persist.tile([P, nt], mybir.dt.float32)

            for t in range(nt):
                ch_tile = io.tile([P, d], mybir.dt.float32, tag="ch")
                nc.sync.dma_start(out=ch_tile[:], in_=ch_t[t])

                q_tile = io.tile([P, d], mybir.dt.float32, tag="q")
                nc.gpsimd.indirect_dma_start(
                    out=q_tile[:],
                    out_offset=None,
                    in_=ht.ap(),
                    in_offset=bass.IndirectOffsetOnAxis(
                        ap=cw_sb[:, t : t + 1], axis=0
                    ),
                )

                d_tile = io.tile([P, d], mybir.dt.float32, tag="d")
                nc.vector.tensor_tensor(
                    out=d_tile[:],
                    in0=ch_tile[:],
                    in1=q_tile[:],
                    op=mybir.AluOpType.subtract,
                )

                sq_tile = scratch.tile([P, d], mybir.dt.float32, tag="sq")
                nc.scalar.activation(
                    out=sq_tile[:],
                    in_=d_tile[:],
                    func=mybir.ActivationFunctionType.Square,
                    accum_out=d2_all[:, t : t + 1],
                )

            # batched tail: one table switch each instead of two per tile
            dist_all = persist.tile([P, nt], mybir.dt.float32)
            nc.scalar.activation(
                out=dist_all[:],
                in_=d2_all[:],
                func=mybir.ActivationFunctionType.Sqrt,
            )
            nc.scalar.activation(
                out=vals_sb[:],
                in_=dist_all[:],
                func=mybir.ActivationFunctionType.Exp,
                scale=1.0 / SMOOTH,
            )

            nc.sync.dma_start(out=vals.ap(), in_=vals_sb[:])
    nc.compile()
    return nc


SUP = 2            # element-tiles per supertile
NSUP = NT // SUP   # 64 supertiles per core
SUPW = SUP * P     # 256 elements per supertile


def build_nc_v4(nt: int = NT, v: int = V, d: int = D) -> bass.Bass:
    """Dedup-gather variant.

    Per supertile (256 sorted elements): gather the <=128 distinct h_t rows
    once (cast to fp16 during the DMA).  Per element-tile, the TensorEngine
    computes d = I^T @ ch - S^T @ W directly in PSUM (host supplies the
    NEGATED one-hot S and the identity), so VectorE is not used at all.
    ScalarE squares+accumulates straight from PSUM; sqrt/exp run batched at
    the end.  ch arrives as fp16 (host cast) to halve its HBM traffic."""
    nsup = nt // SUP
    nc = bacc.Bacc(
        "TRN2", target_bir_lowering=False, debug=False, num_devices=NCORES
    )
    ht = nc.dram_tensor("ht", [v, d], mybir.dt.float32, kind="ExternalInput")
    ch = nc.dram_tensor("ch", [nt * P, d], mybir.dt.float16, kind="ExternalInput")
    # widx[p, s] = p-th (padded) distinct word id of supertile s
    widx = nc.dram_tensor("widx", [P, nsup], mybir.dt.int32, kind="ExternalInput")
    # nsel[t, w, e] = -1 iff element t*128+e's word is the w-th distinct word
    # of supertile t//SUP, else 0
    nsel = nc.dram_tensor("nsel", [nt, P, P], mybir.dt.float16, kind="ExternalInput")
    ident = nc.dram_tensor("ident", [P, P], mybir.dt.float16, kind="ExternalInput")
    vals = nc.dram_tensor("vals", [P, nt], mybir.dt.float32, kind="ExternalOutput")

    ch_ap = ch.ap()      # [nt*P, d]
    nsel_ap = nsel.ap()  # [nt, P, P]

    with tile.TileContext(nc) as tc:
        with (
            tc.tile_pool(name="io", bufs=4) as io,
            tc.tile_pool(name="wpool", bufs=3) as wpool,
            tc.tile_pool(name="spool", bufs=4) as spool,
            tc.tile_pool(name="psum", bufs=4, space="PSUM") as psum,
            tc.tile_pool(name="scratch", bufs=2) as scratch,
            tc.tile_pool(name="persist", bufs=1) as persist,
        ):
            widx_sb = persist.tile([P, nsup], mybir.dt.int32)
            nc.sync.dma_start(out=widx_sb[:], in_=widx.ap())
            ident_sb = persist.tile([P, P], mybir.dt.float16)
            nc.sync.dma_start(out=ident_sb[:], in_=ident.ap())
            vals_sb = persist.tile([P, nt], mybir.dt.float32)
            d2_all = persist.tile([P, nt], mybir.dt.float32)

            for s in range(nsup):
                w_fp = wpool.tile([P, d], mybir.dt.float16, tag="wfp")
                nc.gpsimd.indirect_dma_start(
                    out=w_fp[:],
                    out_offset=None,
                    in_=ht.ap(),
                    in_offset=bass.IndirectOffsetOnAxis(
                        ap=widx_sb[:, s : s + 1], axis=0
                    ),
                )

                # one DMA per supertile for ch ([128, SUP, d]) and nsel
                # ([128, SUP, 128])
                ch_sb = io.tile([P, SUP, d], mybir.dt.float16, tag="ch")
                ch_src = bass.AP(
                    tensor=ch_ap.tensor,
                    offset=s * SUPW * d,
                    ap=[[d, P], [P * d, SUP], [1, d]],
                )
                nc.sync.dma_start(out=ch_sb[:], in_=ch_src)

                ns_sb = spool.tile([P, SUP, P], mybir.dt.float16, tag="nsel")
                ns_src = bass.AP(
                    tensor=nsel_ap.tensor,
                    offset=s * SUP * P * P,
                    ap=[[P, P], [P * P, SUP], [1, P]],
                )
                nc.sync.dma_start(out=ns_sb[:], in_=ns_src)

                # k==0: TensorE also injects ch (d = I^T@ch - S^T@W in PSUM).
                # k==1: VectorE adds ch to the negated gather instead
                # (d = ch + (-S^T@W)), offloading half the inject matmuls.
                q_psums = []
                for k in range(SUP):
                    q_psum = psum.tile([P, d], mybir.dt.float32, tag="q")
                    q_psums.append(q_psum)
                    for h in range(0, d, 512):
                        nc.tensor.matmul(
                            out=q_psum[:, h : h + 512],
                            lhsT=ns_sb[:, k, :],
                            rhs=w_fp[:, h : h + 512],
                            start=True,
                            stop=(k == 1),
                        )
                for h in range(0, d, 512):
                    nc.tensor.matmul(
                        out=q_psums[0][:, h : h + 512],
                        lhsT=ident_sb[:],
                        rhs=ch_sb[:, 0, h : h + 512],
                        start=False,
                        stop=True,
                    )
                d_sb = io.tile([P, d], mybir.dt.float32, tag="dsb")
                nc.vector.tensor_tensor(
                    out=d_sb[:],
                    in0=ch_sb[:, 1, :],
                    in1=q_psums[1][:],
                    op=mybir.AluOpType.add,
                )
                t0 = SUP * s
                sq_tile = scratch.tile([P, d], mybir.dt.float32, tag="sq")
                nc.scalar.activation(
                    out=sq_tile[:],
                    in_=q_psums[0][:],
                    func=mybir.ActivationFunctionType.Square,
                    accum_out=d2_all[:, t0 : t0 + 1],
                )
                sq_tile2 = scratch.tile([P, d], mybir.dt.float32, tag="sq2")
                nc.scalar.activation(
                    out=sq_tile2[:],
                    in_=d_sb[:],
                    func=mybir.ActivationFunctionType.Square,
                    accum_out=d2_all[:, t0 + 1 : t0 + 2],
                )

            dist_all = persist.tile([P, nt], mybir.dt.float32)
            nc.scalar.activation(
                out=dist_all[:],
                in_=d2_all[:],
                func=mybir.ActivationFunctionType.Sqrt,
            )
            nc.scalar.activation(
                out=vals_sb[:],
                in_=dist_all[:],
                func=mybir.ActivationFunctionType.Exp,
                scale=1.0 / SMOOTH,
            )
            nc.sync.dma_start(out=vals.ap(), in_=vals_sb[:])
    nc.compile()
    return nc


def prep_v4(cw_sorted):
    """Per-core supertile metadata. Returns None if any supertile has more
    than 128 distinct words (fall back to per-element gather then)."""
    widx_all, nsel_all = [], []
    neye = -np.eye(P, dtype=np.float16)
    for c in range(NCORES):
        shard = cw_sorted[c * CSH : (c + 1) * CSH]
        widx = np.empty((NSUP, P), np.int32)
        nsel = np.empty((NT, P, P), np.float16)
        for s in range(NSUP):
            seg = shard[s * SUPW : (s + 1) * SUPW]
            uw = np.unique(seg)
            if len(uw) > P:
                return None
            widx[s, : len(uw)] = uw
            widx[s, len(uw) :] = uw[-1]
            rel = np.searchsorted(uw, seg).reshape(SUP, P)
            for k in range(SUP):
                # nsel[t][w, e] = -1 iff rel[k][e] == w
                nsel[SUP * s + k] = neye[:, rel[k]]
        widx_all.append(np.ascontiguousarray(widx.T))
        nsel_all.append(nsel)
    return widx_all, nsel_all


def make_in_maps_v4(h_t, ch_sorted, widx_all, nsel_all):
    ident = np.eye(P, dtype=np.float16)
    in_maps = []
    for c in range(NCORES):
        sl = slice(c * CSH, (c + 1) * CSH)
        in_maps.append(
            {
                "ht": h_t,
                "ch": ch_sorted[sl].astype(np.float16),
                "widx": widx_all[c],
                "nsel": nsel_all[c],
                "ident": ident,
            }
        )
    return in_maps


def make_in_maps(h_t, ch_sorted, cw_sorted):
    in_maps = []
    for c in range(NCORES):
        sl = slice(c * CSH, (c + 1) * CSH)
        in_maps.append(
            {
                "ht": h_t,
                "ch": ch_sorted[sl],
                "cw": np.ascontiguousarray(cw_sorted[sl].reshape(NT, P).T),
            }
        )
    return in_maps


def finish_on_host(vals_sorted, cw_sorted):
    """segment-sum + log_softmax (tiny O(C)+O(V) work)."""
    p = np.bincount(cw_sorted, weights=vals_sorted.astype(np.float64), minlength=V)
    m = p.max()
    lse = m + np.log(np.exp(p - m).sum())
    return (p - lse).astype(np.float32)[None, :]


def _prep(h_t, cache_h, cache_words):
    h_t = np.ascontiguousarray(np.asarray(h_t), dtype=np.float32)
    cache_h = np.ascontiguousarray(np.asarray(cache_h), dtype=np.float32)
    cw = np.asarray(cache_words).astype(np.int32)
    order = np.argsort(cw, kind="stable")
    return h_t, cache_h[order], cw[order]


def run_device(h_t, ch_sorted, cw_sorted, force_v1=False, verbose=False):
    """Compile + run the SPMD program; returns per-element vals (sorted order)."""
    import time as _time

    _t0 = _time.time()
    v4 = None if force_v1 else prep_v4(cw_sorted)
    if v4 is not None:
        nc = build_nc_v4()
        in_maps = make_in_maps_v4(h_t, ch_sorted, *v4)
    else:
        nc = build_nc()
        in_maps = make_in_maps(h_t, ch_sorted, cw_sorted)
    if verbose:
        print(f"[run_device] build+prep: {_time.time() - _t0:.1f}s")
    _t1 = _time.time()
    res = run_bass_kernel_spmd(nc, in_maps, core_ids=list(range(NCORES)))
    if verbose:
        print(f"[run_device] compile+exec: {_time.time() - _t1:.1f}s")
    return np.concatenate([r["vals"].T.reshape(-1) for r in res.results])


def kernel(h_t, cache_h, cache_words):
    h_t, ch_sorted, cw_sorted = _prep(h_t, cache_h, cache_words)
    vals_sorted = run_device(h_t, ch_sorted, cw_sorted)
    return finish_on_host(vals_sorted, cw_sorted)
